# revision 18
# baseline (speedup 1.0000x reference)
"""Trainium2 Bass kernel for MultiHeadEdgeAwareMessagePassing.

Math restructure (validated vs reference, ~1e-3 final rel err incl. bf16):
  logits[i,j,h] = s_q[i,h] + s_k[j,h] + w[i,j]*c1[h] + c0[h]   (valid j: w>0)
  alpha = softmax_j(logits) * w
s_q, c0 are constant over j and cancel in the softmax; bk's contribution to
s_k scales numerator and denominator equally and cancels too. With
g[j,h] = exp(h[j]@a_k[h]), a_k[h] = u_k[h] @ Wk[h-block], v = h@Wv^T + bv:
  msg[i,h,:] = Num_h[i,:] / Den_h[i]
  Num_h = W1^T (g_h*v_h)
  Den_h = mask^T g_h + c1_h (W1^T g_h)
where mask=[w>0], W1=relu(w)  (exp(c1 w) ~= 1 + c1 w, |c1 w| < 0.02; the
dropped quadratic term changes the final output by ~3e-6 relative).

Sharding: destination rows i split across 8 cores (384 rows each). Each core
reads its [3072, 384] slice of w^T plus replicated h^T and the small weights.
Host-side transposes are layout prep only; all compute runs on device.
"""

import numpy as np

N = 3072
D = 256
H = 4
DH = 64
DE = 8
NCORES = 8
ISLICE = N // NCORES  # 384
NSUB = ISLICE // 128  # 3
CJT = 4               # j-tiles per chunk
NCH = N // (128 * CJT)  # 6 chunks

_cache = {}


def _build_bass():
    import concourse.bass as bass
    import concourse.tile as tile
    from concourse import bacc, mybir
    from concourse.bass import ts
    from concourse.masks import make_identity

    dt = mybir.dt
    AF = mybir.ActivationFunctionType
    OP = mybir.AluOpType

    nc = bacc.Bacc("TRN2", target_bir_lowering=False, debug=False,
                   num_devices=NCORES)

    wt_d = nc.dram_tensor("wt", [N, ISLICE], dt.float32, kind="ExternalInput")
    ht_d = nc.dram_tensor("ht", [D, N], dt.float32, kind="ExternalInput")
    hs_d = nc.dram_tensor("hs", [ISLICE, D], dt.float32, kind="ExternalInput")
    Wk_d = nc.dram_tensor("Wk", [D, D], dt.float32, kind="ExternalInput")
    WvT_d = nc.dram_tensor("WvT", [D, D], dt.float32, kind="ExternalInput")
    WoT_d = nc.dram_tensor("WoT", [D, D], dt.float32, kind="ExternalInput")
    u_d = nc.dram_tensor("u", [H, 2 * DH + DE], dt.float32, kind="ExternalInput")
    bv_d = nc.dram_tensor("bv", [D], dt.float32, kind="ExternalInput")
    bo_d = nc.dram_tensor("bo", [D], dt.float32, kind="ExternalInput")
    Wew_d = nc.dram_tensor("Wew", [H * DE, 1], dt.float32, kind="ExternalInput")
    gam_d = nc.dram_tensor("gamma", [D], dt.float32, kind="ExternalInput")
    bet_d = nc.dram_tensor("beta", [D], dt.float32, kind="ExternalInput")
    out_d = nc.dram_tensor("out", [ISLICE, D], dt.float32, kind="ExternalOutput")

    bf = dt.bfloat16
    f32 = dt.float32

    with tile.TileContext(nc) as tc:
        with (
            tc.tile_pool(name="consts", bufs=1) as consts,
            tc.tile_pool(name="wtp", bufs=3) as wtp,
            tc.tile_pool(name="htf", bufs=3) as htfp,
            tc.tile_pool(name="elem", bufs=3) as elem,
            tc.tile_pool(name="rhsp", bufs=4) as rhsp,
            tc.tile_pool(name="gp", bufs=3) as gp,
            tc.tile_pool(name="small", bufs=4) as small,
            tc.tile_pool(name="outp", bufs=2) as outp,
            tc.tile_pool(name="acc", bufs=1, space="PSUM") as accp,
            tc.tile_pool(name="pre4", bufs=2, space="PSUM") as pre4,
            tc.tile_pool(name="presk", bufs=1, space="PSUM") as presk,
        ):
            # ---- small constants: f32 via the ACT HWDGE ring (parallel to
            # sync's wt/ht stream), cast to bf16 on then-idle DVE ----
            stg_wv = consts.tile([128, 2, D], f32, tag="stgwv")
            nc.scalar.dma_start(stg_wv,
                                WvT_d.ap().rearrange("(a p) n -> p a n", p=128))
            stg_wk = consts.tile([DH, H, D], f32, tag="stgwk")
            nc.scalar.dma_start(
                stg_wk, bass.AP(tensor=Wk_d, offset=0,
                                ap=[[D, DH], [DH * D, H], [1, D]]))
            stg_sm = consts.tile([DH, 3 * H + 2 * H + 2], f32, tag="stgsm")
            nc.scalar.dma_start(
                stg_sm[:, 0:H],
                bass.AP(tensor=u_d, offset=DH, ap=[[1, DH], [2 * DH + DE, H]]))
            nc.scalar.dma_start(
                stg_sm[0:DE, H:2 * H],
                bass.AP(tensor=u_d, offset=2 * DH,
                        ap=[[1, DE], [2 * DH + DE, H]]))
            nc.scalar.dma_start(
                stg_sm[0:DE, 2 * H:3 * H],
                bass.AP(tensor=Wew_d, offset=0, ap=[[1, DE], [DE, H]]))
            stg_bv = consts.tile([1, D], f32, tag="stgbv")
            nc.scalar.dma_start(stg_bv,
                                bv_d.ap().rearrange("(o f) -> o f", o=1))

            rhs_wv = consts.tile([128, 2, D], bf, tag="rhswv")
            nc.vector.tensor_copy(rhs_wv, stg_wv)
            Wk2 = consts.tile([DH, H, D], bf, tag="wk2")
            nc.vector.tensor_copy(Wk2, stg_wk)
            sm_bf = consts.tile([DH, 3 * H], bf, tag="smbf")
            nc.vector.tensor_copy(sm_bf, stg_sm[:, 0:3 * H])
            u4 = sm_bf[:, 0:H]
            ue4 = sm_bf[0:DE, H:2 * H]
            Wew2 = sm_bf[0:DE, 2 * H:3 * H]
            bv_row = consts.tile([1, D], bf, tag="bvrow")
            nc.vector.tensor_copy(bv_row, stg_bv)

            ones_sb = consts.tile([1, 128], bf, tag="ones")
            nc.vector.memset(ones_sb, 1.0)

            # ------------- epilogue constants (SWDGE, off critical path) -----
            WoT_sb = consts.tile([128, 2, D], bf, tag="wot")
            nc.gpsimd.dma_start(WoT_sb,
                                WoT_d.ap().rearrange("(a p) n -> p a n", p=128))
            bo_row = consts.tile([1, 256], bf, tag="borow")
            nc.gpsimd.dma_start(bo_row, bo_d.ap().rearrange("(o f) -> o f", o=1))
            ident = consts.tile([128, 128], bf, tag="ident")
            make_identity(nc, ident)
            gam_sb = consts.tile([128, D], f32, tag="gam")
            nc.gpsimd.dma_start(
                gam_sb, bass.AP(tensor=gam_d, offset=0, ap=[[0, 128], [1, D]]))
            bet_sb = consts.tile([128, D], f32, tag="bet")
            nc.gpsimd.dma_start(
                bet_sb, bass.AP(tensor=bet_d, offset=0, ap=[[0, 128], [1, D]]))
            eps_sb = consts.tile([128, 1], f32, tag="eps")
            nc.vector.memset(eps_sb, 1e-5)

            # ------------- setup matmuls -------------
            # a_k^T[dm, h] = sum_d Wk[h*64+d, dm] u_k[h, d]
            rhs_ak = consts.tile([128, 2, H], bf, tag="rhsak")
            for b in range(2):
                ps_ak = presk.tile([128, H], f32, tag="sk4")
                for h in range(H):
                    nc.tensor.matmul(ps_ak[:, h:h + 1],
                                     Wk2[:, h, 128 * b:128 * (b + 1)],
                                     u4[:, h:h + 1], start=True, stop=True,
                                     skip_group_check=True)
                nc.vector.tensor_copy(rhs_ak[:, b, :], ps_ak)

            # c1[h] = sum_d We_w[h*8+d] u_e[h, d], broadcast to partitions
            ps_c1 = presk.tile([1, H], f32, tag="sk4")
            for h in range(H):
                nc.tensor.matmul(ps_c1[:, h:h + 1], Wew2[:, h:h + 1],
                                 ue4[:, h:h + 1], start=True, stop=True,
                                 skip_group_check=True)
            c1row = consts.tile([1, H], bf, tag="c1row")
            nc.vector.tensor_copy(c1row, ps_c1)
            ps_c1b = presk.tile([128, H], f32, tag="sk4")
            nc.tensor.matmul(ps_c1b, ones_sb, c1row, start=True, stop=True)
            c1b = consts.tile([128, H], f32, tag="c1b")
            nc.vector.tensor_copy(c1b, ps_c1b)

            # ---------------- persistent accumulators ----------------
            # cols 0:256 = W1.gV, 256:260 = W1.g, 260:264 = mask.g
            psA = [accp.tile([128, 264], f32, tag=f"A{s}", name=f"psA{s}")
                   for s in range(NSUB)]

            ht_sb = consts.tile([128, 2, N], bf, tag="ht")
            ht_re = ht_d.ap().rearrange("(a p) n -> p a n", p=128)

            # ---------------- main loop ----------------
            for ch in range(NCH):
                wt4 = wtp.tile([128, CJT, ISLICE], f32, tag="wt")
                nc.sync.dma_start(
                    wt4, wt_d[ts(ch, 128 * CJT), :].rearrange(
                        "(j p) i -> p j i", p=128))

                # h^T chunk: f32 via HWDGE, cast to bf16 on ACT/DVE alternately
                htf = htfp.tile([128, 2, 128 * CJT], f32, tag="htf")
                nc.sync.dma_start(htf, ht_re[:, :, ts(ch, 128 * CJT)])
                htc = ht_sb[:, :, ts(ch, 128 * CJT)]
                if ch % 2 == 0:
                    nc.scalar.copy(htc, htf)
                else:
                    nc.vector.tensor_copy(htc, htf)

                W1c = elem.tile([128, CJT, ISLICE], bf, tag="W1")
                nc.scalar.activation(W1c, wt4, AF.Relu)
                mskc = elem.tile([128, CJT, ISLICE], bf, tag="msk")
                nc.vector.tensor_scalar(mskc, wt4, 0.0, None, op0=OP.is_gt)

                # --- v and s_k for the CJT j-tiles of this chunk ---
                ps_v4 = pre4.tile([128, CJT, 256], f32, tag="v4")
                ps_sk4 = presk.tile([128, CJT, H], f32, tag="sk4")
                for jm in range(CJT):
                    jt = ch * CJT + jm
                    for a in range(2):
                        nc.tensor.matmul(ps_v4[:, jm, :],
                                         ht_sb[:, a, ts(jt, 128)],
                                         rhs_wv[:, a, :],
                                         start=(a == 0), stop=False)
                        nc.tensor.matmul(ps_sk4[:, jm, :],
                                         ht_sb[:, a, ts(jt, 128)],
                                         rhs_ak[:, a, :],
                                         start=(a == 0), stop=(a == 1))
                    nc.tensor.matmul(ps_v4[:, jm, :], ones_sb, bv_row,
                                     start=False, stop=True)

                g32 = gp.tile([128, CJT, H], f32, tag="g32")
                nc.scalar.activation(g32, ps_sk4, AF.Exp)

                rhs4 = rhsp.tile([128, CJT, 260], bf, tag="rhsbig")
                g32b = bass.AP(tensor=g32.tensor, offset=g32.offset,
                               ap=[g32.ap[0], g32.ap[1], g32.ap[2], [0, DH]])
                nc.vector.tensor_tensor(
                    out=rhs4[:, :, 0:256].rearrange(
                        "p j (h d) -> p j h d", h=H),
                    in0=ps_v4.rearrange("p j (h d) -> p j h d", h=H),
                    in1=g32b, op=OP.mult)
                nc.vector.tensor_copy(rhs4[:, :, 256:260], g32)

                st = (ch == 0)
                sp = (ch == NCH - 1)
                for jm in range(CJT):
                    for s in range(NSUB):
                        sl = ts(s, 128)
                        nc.tensor.matmul(psA[s][:, 0:260], W1c[:, jm, sl],
                                         rhs4[:, jm, :], start=st, stop=sp,
                                         skip_group_check=True)
                        nc.tensor.matmul(psA[s][:, 260:264], mskc[:, jm, sl],
                                         rhs4[:, jm, 256:260], start=st, stop=sp,
                                         skip_group_check=True)

            # ---------------- epilogue ----------------
            for s in range(NSUB):
                dg = small.tile([128, H], f32, tag="dg")
                nc.vector.tensor_copy(dg, psA[s][:, 256:260])
                den = small.tile([128, H], f32, tag="den")
                nc.vector.tensor_mul(den, c1b, dg)
                nc.vector.tensor_add(den, den, psA[s][:, 260:264])
                rden = small.tile([128, H], f32, tag="rden")
                nc.vector.reciprocal(rden, den)

                msg = outp.tile([128, D], bf, tag="msg")
                for h in range(H):
                    hsl = slice(h * DH, (h + 1) * DH)
                    nc.vector.tensor_scalar(msg[:, hsl], psA[s][:, hsl],
                                            rden[:, h:h + 1], None, op0=OP.mult)

                msgT = outp.tile([128, 2, 128], bf, tag="msgT")
                for b in range(2):
                    ps_t = pre4.tile([128, 128], bf, tag="v4")
                    nc.tensor.transpose(ps_t, msg[:, ts(b, 128)], ident)
                    nc.vector.tensor_copy(msgT[:, b, :], ps_t)

                ps_o = pre4.tile([128, D], f32, tag="v4")
                nc.tensor.matmul(ps_o, msgT[:, 0, :], WoT_sb[:, 0, :],
                                 start=True, stop=False)
                nc.tensor.matmul(ps_o, msgT[:, 1, :], WoT_sb[:, 1, :],
                                 start=False, stop=False)
                nc.tensor.matmul(ps_o, ones_sb, bo_row, start=False, stop=True)

                x = outp.tile([128, D], f32, tag="x")
                hseg = outp.tile([128, D], f32, tag="hseg")
                nc.sync.dma_start(hseg, hs_d[ts(s, 128), :])
                nc.vector.tensor_add(x, ps_o, hseg)

                stats = small.tile([128, 6], f32, tag="stats")
                nc.vector.bn_stats(out=stats, in_=x)
                mv = small.tile([128, 2], f32, tag="mv")
                nc.vector.bn_aggr(out=mv, in_=stats)
                sd = small.tile([128, 1], f32, tag="sd")
                nc.scalar.activation(sd, mv[:, 1:2], AF.Sqrt, bias=eps_sb)
                rstd = small.tile([128, 1], f32, tag="rstd")
                nc.vector.reciprocal(rstd, sd)

                y = outp.tile([128, D], f32, tag="y")
                nc.vector.tensor_scalar(y, x, mv[:, 0:1], rstd,
                                        op0=OP.subtract, op1=OP.mult)
                ot = outp.tile([128, D], f32, tag="ot")
                nc.vector.tensor_mul(ot, y, gam_sb)
                nc.vector.tensor_add(ot, ot, bet_sb)
                nc.sync.dma_start(out_d[ts(s, 128), :], ot)

    nc.compile()
    return nc


def _make_in_maps(h, w, Wk, Wv, bv, We_w, u, Wo, bo, gamma, beta, **_unused):
    f = np.float32
    h = np.ascontiguousarray(h, dtype=f)
    wT = np.ascontiguousarray(np.asarray(w, dtype=f).T)
    common = {
        "ht": np.ascontiguousarray(h.T),
        "Wk": np.ascontiguousarray(Wk, dtype=f),
        "WvT": np.ascontiguousarray(np.asarray(Wv, dtype=f).T),
        "WoT": np.ascontiguousarray(np.asarray(Wo, dtype=f).T),
        "u": np.ascontiguousarray(u, dtype=f),
        "bv": np.ascontiguousarray(bv, dtype=f),
        "bo": np.ascontiguousarray(bo, dtype=f),
        "Wew": np.ascontiguousarray(We_w, dtype=f),
        "gamma": np.ascontiguousarray(gamma, dtype=f),
        "beta": np.ascontiguousarray(beta, dtype=f),
    }
    in_maps = []
    for c in range(NCORES):
        sl = slice(c * ISLICE, (c + 1) * ISLICE)
        m = dict(common)
        m["wt"] = np.ascontiguousarray(wT[:, sl])
        m["hs"] = np.ascontiguousarray(h[sl, :])
        in_maps.append(m)
    return in_maps


def kernel(**inputs):
    from concourse.bass_utils import run_bass_kernel_spmd

    if "nc" not in _cache:
        _cache["nc"] = _build_bass()
    nc = _cache["nc"]

    in_maps = _make_in_maps(**inputs)
    res = run_bass_kernel_spmd(nc, in_maps, core_ids=list(range(NCORES)))
    out = np.concatenate([r["out"] for r in res.results], axis=0)
    return np.ascontiguousarray(out, dtype=np.float32)


# revision 24
# speedup vs baseline: 1.1646x; 1.1646x over previous
"""Trainium2 Bass kernel for MultiHeadEdgeAwareMessagePassing.

Math restructure (validated vs reference, ~1e-3 final rel err incl. bf16):
  logits[i,j,h] = s_q[i,h] + s_k[j,h] + w[i,j]*c1[h] + c0[h]   (valid j: w>0)
  alpha = softmax_j(logits) * w
s_q, c0 are constant over j and cancel in the softmax; bk's contribution to
s_k scales numerator and denominator equally and cancels too. With
g[j,h] = exp(h[j]@a_k[h]), a_k[h] = u_k[h] @ Wk[h-block], v = h@Wv^T + bv:
  msg[i,h,:] = Num_h[i,:] / Den_h[i]
  Num_h = W1^T (g_h*v_h)
  Den_h = mask^T g_h + c1_h (W1^T g_h)
where mask=[w>0], W1=relu(w)  (exp(c1 w) ~= 1 + c1 w, |c1 w| < 0.02; the
dropped quadratic term changes the final output by ~3e-6 relative).

Sharding: destination rows i split across 8 cores (384 rows each). Each core
reads its [3072, 384] slice of w^T plus replicated h^T and the small weights.
Host-side transposes are layout prep only; all compute runs on device.
"""

import numpy as np

N = 3072
D = 256
H = 4
DH = 64
DE = 8
NCORES = 8
ISLICE = N // NCORES  # 384
NSUB = ISLICE // 128  # 3
CJT = 4               # j-tiles per chunk
NCH = N // (128 * CJT)  # 6 chunks

_cache = {}


def _build_bass():
    import concourse.bass as bass
    import concourse.tile as tile
    from concourse import bacc, mybir
    from concourse.bass import ts
    from concourse.masks import make_identity

    dt = mybir.dt
    AF = mybir.ActivationFunctionType
    OP = mybir.AluOpType

    nc = bacc.Bacc("TRN2", target_bir_lowering=False, debug=False,
                   num_devices=NCORES)

    wt_d = nc.dram_tensor("wt", [N, ISLICE], dt.float32, kind="ExternalInput")
    ht_d = nc.dram_tensor("ht", [D, N], dt.float32, kind="ExternalInput")
    hs_d = nc.dram_tensor("hs", [ISLICE, D], dt.float32, kind="ExternalInput")
    setup_d = nc.dram_tensor("setup", [128, 1804], dt.float32,
                             kind="ExternalInput")
    WoT_d = nc.dram_tensor("WoT", [D, D], dt.float32, kind="ExternalInput")
    bo_d = nc.dram_tensor("bo", [D], dt.float32, kind="ExternalInput")
    gam_d = nc.dram_tensor("gamma", [D], dt.float32, kind="ExternalInput")
    bet_d = nc.dram_tensor("beta", [D], dt.float32, kind="ExternalInput")
    out_d = nc.dram_tensor("out", [ISLICE, D], dt.float32, kind="ExternalOutput")

    bf = dt.bfloat16
    f32 = dt.float32

    with tile.TileContext(nc) as tc:
        with (
            tc.tile_pool(name="consts", bufs=1) as consts,
            tc.tile_pool(name="wtp", bufs=3) as wtp,
            tc.tile_pool(name="htf", bufs=3) as htfp,
            tc.tile_pool(name="elem", bufs=3) as elem,
            tc.tile_pool(name="rhsp", bufs=4) as rhsp,
            tc.tile_pool(name="gp", bufs=3) as gp,
            tc.tile_pool(name="small", bufs=4) as small,
            tc.tile_pool(name="outp", bufs=2) as outp,
            tc.tile_pool(name="acc", bufs=1, space="PSUM") as accp,
            tc.tile_pool(name="pre4", bufs=2, space="PSUM") as pre4,
            tc.tile_pool(name="presk", bufs=1, space="PSUM") as presk,
        ):
            # ---- small constants: host-packed into one array, one sync DMA
            # (first on the queue), one DVE cast ----
            stg = consts.tile([128, 1804], f32, tag="stg")
            nc.sync.dma_start(stg, setup_d.ap())
            sbf = consts.tile([128, 1548], bf, tag="sbf")
            nc.vector.tensor_copy(sbf, stg[:, 0:1548])
            bv_row = consts.tile([1, D], bf, tag="bvrow")
            nc.vector.tensor_copy(bv_row, stg[0:1, 1548:1804])

            rhs_wv = sbf[:, 0:512].rearrange("p (a n) -> p a n", a=2)

            ones_sb = consts.tile([1, 128], bf, tag="ones")
            nc.vector.memset(ones_sb, 1.0)

            # ------------- epilogue constants (SWDGE, off critical path) -----
            WoT_sb = consts.tile([128, 2, D], bf, tag="wot")
            nc.gpsimd.dma_start(WoT_sb,
                                WoT_d.ap().rearrange("(a p) n -> p a n", p=128))
            bo_row = consts.tile([1, 256], bf, tag="borow")
            nc.gpsimd.dma_start(bo_row, bo_d.ap().rearrange("(o f) -> o f", o=1))
            gam_sb = consts.tile([128, D], f32, tag="gam")
            nc.gpsimd.dma_start(
                gam_sb, bass.AP(tensor=gam_d, offset=0, ap=[[0, 128], [1, D]]))
            bet_sb = consts.tile([128, D], f32, tag="bet")
            nc.gpsimd.dma_start(
                bet_sb, bass.AP(tensor=bet_d, offset=0, ap=[[0, 128], [1, D]]))
            ident = consts.tile([128, 128], bf, tag="ident")
            make_identity(nc, ident)
            eps_sb = consts.tile([128, 1], f32, tag="eps")
            nc.vector.memset(eps_sb, 1e-5)

            # ------------- setup matmuls -------------
            # a_k^T[dm, h] = sum_d Wk[h*64+d, dm] u_k[h, d]
            rhs_ak = consts.tile([128, 2, H], bf, tag="rhsak")
            for b in range(2):
                ps_ak = presk.tile([128, H], f32, tag="sk4")
                for h in range(H):
                    nc.tensor.matmul(
                        ps_ak[:, h:h + 1],
                        sbf[0:DH, 512 + h * 256 + 128 * b:
                            512 + h * 256 + 128 * (b + 1)],
                        sbf[0:DH, 1536 + h:1537 + h],
                        start=True, stop=True, skip_group_check=True)
                nc.vector.tensor_copy(rhs_ak[:, b, :], ps_ak)

            # c1[h] = sum_d We_w[h*8+d] u_e[h, d], broadcast to partitions
            ps_c1 = presk.tile([1, H], f32, tag="sk4")
            for h in range(H):
                nc.tensor.matmul(ps_c1[:, h:h + 1],
                                 sbf[0:DE, 1544 + h:1545 + h],
                                 sbf[0:DE, 1540 + h:1541 + h],
                                 start=True, stop=True,
                                 skip_group_check=True)
            c1row = consts.tile([1, H], bf, tag="c1row")
            nc.vector.tensor_copy(c1row, ps_c1)
            ps_c1b = presk.tile([128, H], f32, tag="sk4")
            nc.tensor.matmul(ps_c1b, ones_sb, c1row, start=True, stop=True)
            c1b = consts.tile([128, H], f32, tag="c1b")
            nc.vector.tensor_copy(c1b, ps_c1b)

            # ---------------- persistent accumulators ----------------
            # cols 0:256 = W1.gV, 256:260 = W1.g, 260:264 = mask.g
            psA = [accp.tile([128, 264], f32, tag=f"A{s}", name=f"psA{s}")
                   for s in range(NSUB)]

            ht_sb = consts.tile([128, 2, N], bf, tag="ht")
            ht_re = ht_d.ap().rearrange("(a p) n -> p a n", p=128)

            # ---------------- main loop ----------------
            for ch in range(NCH):
                wt4 = wtp.tile([128, CJT, ISLICE], f32, tag="wt")
                nc.sync.dma_start(
                    wt4, wt_d[ts(ch, 128 * CJT), :].rearrange(
                        "(j p) i -> p j i", p=128))

                # h^T chunk: f32 via HWDGE, cast to bf16 on ACT/DVE alternately
                htf = htfp.tile([128, 2, 128 * CJT], f32, tag="htf")
                nc.sync.dma_start(htf, ht_re[:, :, ts(ch, 128 * CJT)])
                htc = ht_sb[:, :, ts(ch, 128 * CJT)]
                if ch % 2 == 0:
                    nc.scalar.copy(htc, htf)
                else:
                    nc.vector.tensor_copy(htc, htf)

                W1c = elem.tile([128, CJT, ISLICE], bf, tag="W1")
                nc.scalar.activation(W1c, wt4, AF.Relu)
                mskc = elem.tile([128, CJT, ISLICE], bf, tag="msk")
                nc.vector.tensor_scalar(mskc, wt4, 0.0, None, op0=OP.is_gt)

                # --- v and s_k for the CJT j-tiles of this chunk ---
                ps_v4 = pre4.tile([128, CJT, 256], f32, tag="v4")
                ps_sk4 = presk.tile([128, CJT, H], f32, tag="sk4")
                for jm in range(CJT):
                    jt = ch * CJT + jm
                    for a in range(2):
                        nc.tensor.matmul(ps_v4[:, jm, :],
                                         ht_sb[:, a, ts(jt, 128)],
                                         rhs_wv[:, a, :],
                                         start=(a == 0), stop=False)
                        nc.tensor.matmul(ps_sk4[:, jm, :],
                                         ht_sb[:, a, ts(jt, 128)],
                                         rhs_ak[:, a, :],
                                         start=(a == 0), stop=(a == 1))
                    nc.tensor.matmul(ps_v4[:, jm, :], ones_sb, bv_row,
                                     start=False, stop=True)

                g32 = gp.tile([128, CJT, H], f32, tag="g32")
                nc.scalar.activation(g32, ps_sk4, AF.Exp)

                rhs4 = rhsp.tile([128, CJT, 260], bf, tag="rhsbig")
                g32b = bass.AP(tensor=g32.tensor, offset=g32.offset,
                               ap=[g32.ap[0], g32.ap[1], g32.ap[2], [0, DH]])
                nc.vector.tensor_tensor(
                    out=rhs4[:, :, 0:256].rearrange(
                        "p j (h d) -> p j h d", h=H),
                    in0=ps_v4.rearrange("p j (h d) -> p j h d", h=H),
                    in1=g32b, op=OP.mult)
                nc.vector.tensor_copy(rhs4[:, :, 256:260], g32)

                st = (ch == 0)
                sp = (ch == NCH - 1)
                for jm in range(CJT):
                    for s in range(NSUB):
                        sl = ts(s, 128)
                        nc.tensor.matmul(psA[s][:, 0:260], W1c[:, jm, sl],
                                         rhs4[:, jm, :], start=st, stop=sp,
                                         skip_group_check=True)
                        nc.tensor.matmul(psA[s][:, 260:264], mskc[:, jm, sl],
                                         rhs4[:, jm, 256:260], start=st, stop=sp,
                                         skip_group_check=True)

            # ---------------- epilogue ----------------
            for s in range(NSUB):
                dg = small.tile([128, H], f32, tag="dg")
                nc.vector.tensor_copy(dg, psA[s][:, 256:260])
                den = small.tile([128, H], f32, tag="den")
                nc.vector.tensor_mul(den, c1b, dg)
                nc.vector.tensor_add(den, den, psA[s][:, 260:264])
                rden = small.tile([128, H], f32, tag="rden")
                nc.vector.reciprocal(rden, den)

                msg = outp.tile([128, D], bf, tag="msg")
                for h in range(H):
                    hsl = slice(h * DH, (h + 1) * DH)
                    nc.vector.tensor_scalar(msg[:, hsl], psA[s][:, hsl],
                                            rden[:, h:h + 1], None, op0=OP.mult)

                msgT = outp.tile([128, 2, 128], bf, tag="msgT")
                for b in range(2):
                    ps_t = pre4.tile([128, 128], bf, tag="v4")
                    nc.tensor.transpose(ps_t, msg[:, ts(b, 128)], ident)
                    nc.vector.tensor_copy(msgT[:, b, :], ps_t)

                ps_o = pre4.tile([128, D], f32, tag="v4")
                nc.tensor.matmul(ps_o, msgT[:, 0, :], WoT_sb[:, 0, :],
                                 start=True, stop=False)
                nc.tensor.matmul(ps_o, msgT[:, 1, :], WoT_sb[:, 1, :],
                                 start=False, stop=False)
                nc.tensor.matmul(ps_o, ones_sb, bo_row, start=False, stop=True)

                x = outp.tile([128, D], f32, tag="x")
                hseg = outp.tile([128, D], f32, tag="hseg")
                nc.sync.dma_start(hseg, hs_d[ts(s, 128), :])
                nc.vector.tensor_add(x, ps_o, hseg)

                stats = small.tile([128, 6], f32, tag="stats")
                nc.vector.bn_stats(out=stats, in_=x)
                mv = small.tile([128, 2], f32, tag="mv")
                nc.vector.bn_aggr(out=mv, in_=stats)
                sd = small.tile([128, 1], f32, tag="sd")
                nc.scalar.activation(sd, mv[:, 1:2], AF.Sqrt, bias=eps_sb)
                rstd = small.tile([128, 1], f32, tag="rstd")
                nc.vector.reciprocal(rstd, sd)

                y = outp.tile([128, D], f32, tag="y")
                nc.vector.tensor_scalar(y, x, mv[:, 0:1], rstd,
                                        op0=OP.subtract, op1=OP.mult)
                ot = outp.tile([128, D], f32, tag="ot")
                nc.vector.tensor_mul(ot, y, gam_sb)
                nc.vector.tensor_add(ot, ot, bet_sb)
                nc.sync.dma_start(out_d[ts(s, 128), :], ot)

    nc.compile()
    return nc


def _make_in_maps(h, w, Wk, Wv, bv, We_w, u, Wo, bo, gamma, beta, **_unused):
    f = np.float32
    h = np.ascontiguousarray(h, dtype=f)
    wT = np.ascontiguousarray(np.asarray(w, dtype=f).T)

    # packed setup array: [128, 1804] f32 (all matmul zones at part base 0)
    # cols 0:512     WvT[a*128+p, n] at col a*256+n
    # cols 512:1536  Wk[h*64+d, n] at (part d, col 512+h*256+n)
    # cols 1536:1540 u_k[h, d] at (part d, col 1536+h)
    # cols 1540:1544 u_e[h, d] at (part d, col 1540+h), d<8
    # cols 1544:1548 We_w[h*8+d, 0] likewise
    # cols 1548:1804 bv on partition 0
    setup = np.zeros((128, 1804), f)
    WvT = np.asarray(Wv, dtype=f).T
    setup[:, 0:512] = WvT.reshape(2, 128, D).transpose(1, 0, 2).reshape(128, 512)
    Wk = np.asarray(Wk, dtype=f)
    u = np.asarray(u, dtype=f)
    We_w = np.asarray(We_w, dtype=f)
    for hh in range(H):
        setup[0:DH, 512 + hh * 256:512 + (hh + 1) * 256] = \
            Wk[hh * DH:(hh + 1) * DH, :]
        setup[0:DH, 1536 + hh] = u[hh, DH:2 * DH]
        setup[0:DE, 1540 + hh] = u[hh, 2 * DH:2 * DH + DE]
        setup[0:DE, 1544 + hh] = We_w[hh * DE:(hh + 1) * DE, 0]
    setup[0, 1548:1804] = np.asarray(bv, dtype=f)

    common = {
        "ht": np.ascontiguousarray(h.T),
        "setup": setup,
        "WoT": np.ascontiguousarray(np.asarray(Wo, dtype=f).T),
        "bo": np.ascontiguousarray(bo, dtype=f),
        "gamma": np.ascontiguousarray(gamma, dtype=f),
        "beta": np.ascontiguousarray(beta, dtype=f),
    }
    in_maps = []
    for c in range(NCORES):
        sl = slice(c * ISLICE, (c + 1) * ISLICE)
        m = dict(common)
        m["wt"] = np.ascontiguousarray(wT[:, sl])
        m["hs"] = np.ascontiguousarray(h[sl, :])
        in_maps.append(m)
    return in_maps


def kernel(**inputs):
    from concourse.bass_utils import run_bass_kernel_spmd

    if "nc" not in _cache:
        _cache["nc"] = _build_bass()
    nc = _cache["nc"]

    in_maps = _make_in_maps(**inputs)
    res = run_bass_kernel_spmd(nc, in_maps, core_ids=list(range(NCORES)))
    out = np.concatenate([r["out"] for r in res.results], axis=0)
    return np.ascontiguousarray(out, dtype=np.float32)
